# revision 1
# baseline (speedup 1.0000x reference)
"""Trainium2 Bass kernel for Linformer-style sparse attention.

Problem shapes (hardcoded): B=4, S=4096, D=1024, H=16, HD=64, LK=256.

Sharding (8 cores): core c -> (batch b = c//2, sequence half = c%2).
Each core:
  - computes Q/K/V for its 2048 rows (all heads),
  - computes partial [Kp^T; Vp^T] = (K|V)^T @ E^T over its rows,
  - pair AllReduce ([0,1],[2,3],[4,5],[6,7]) completes Kp/Vp (1 MiB bf16),
  - attention (softmax over LK=256) + output projection for its own rows,
  - writes its [2048, 1024] slice of the output directly (no final collective).

All matmuls run in bf16 (f32->bf16 casts happen inside SWDGE DMAs); X^T and
E^T are produced with the XBAR dma_start_transpose (single HWDGE ring - two
concurrent transpose rings corrupt data). Q^T matmul chunks are interleaved
into the E^T-transpose-bound partials window to keep the PE dense, and the
AllReduce is covered by the tail Q chunk + readbacks.
"""

import sys

sys.path.insert(0, "/opt/trn_rl_repo")

from contextlib import ExitStack

import numpy as np

from concourse import bacc, bass_utils, mybir, tile
from concourse.masks import make_identity

B, S, D = 4, 4096, 1024
H, HD, LK = 16, 64, 256
SL = S // 2            # local sequence rows per core
P = 128
NSC = SL // P          # 16 s-chunks of 128
NDC = D // P           # 8 d-chunks of 128
NSN = SL // 512        # 4 s-chunks of 512
f32 = mybir.dt.float32
bf16 = mybir.dt.bfloat16
PAIRS = [[0, 1], [2, 3], [4, 5], [6, 7]]


def _build(include_biases: bool, debug: bool = False):
    nc = bacc.Bacc("TRN2", target_bir_lowering=False, num_devices=8)

    X_e = nc.declare_dram_parameter("X", [SL, D], f32, isOutput=False)
    mask_e = nc.declare_dram_parameter("mask", [SL], f32, isOutput=False)
    Wq_e = nc.declare_dram_parameter("Wq", [D, D], f32, isOutput=False)
    bq_e = nc.declare_dram_parameter("bq", [D], f32, isOutput=False)
    Wk_e = nc.declare_dram_parameter("Wk", [D, D], f32, isOutput=False)
    bk_e = nc.declare_dram_parameter("bk", [D], f32, isOutput=False)
    Wv_e = nc.declare_dram_parameter("Wv", [D, D], f32, isOutput=False)
    bv_e = nc.declare_dram_parameter("bv", [D], f32, isOutput=False)
    E_e = nc.declare_dram_parameter("E", [H, LK, SL], f32, isOutput=False)
    Wo_e = nc.declare_dram_parameter("Wo", [D, D], f32, isOutput=False)
    bo_e = nc.declare_dram_parameter("bo", [D], f32, isOutput=False)
    out_e = nc.declare_dram_parameter("out", [SL, D], f32, isOutput=True)

    ebf_d = nc.dram_tensor("ebf", [H, LK, SL], bf16, kind="Internal")
    # AllReduce bounce (bf16): per head [KpT ; VpT] stacked [128, 256] flat
    cc_in = nc.dram_tensor("cc_in", [H, P * LK], bf16, kind="Internal")
    cc_out = nc.dram_tensor("cc_out", [H, P * LK], bf16, kind="Internal")

    with tile.TileContext(nc) as tc:
        ctx = ExitStack()
        with ctx:
            const_pool = ctx.enter_context(tc.tile_pool(name="consts", bufs=1))

            # ---------------- constants ----------------
            m_sb = const_pool.tile([P, NSC], f32, name="m_sb")
            nc.sync.dma_start(m_sb[:], mask_e.ap().rearrange("(o p) -> p o", p=P))
            bq_sb = const_pool.tile([P, NDC], f32, name="bq_sb")
            nc.sync.dma_start(bq_sb[:], bq_e.ap().rearrange("(o p) -> p o", p=P))
            bo_bc = const_pool.tile([P, D], bf16, name="bo_bc")
            nc.gpsimd.dma_start(out=bo_bc[:], in_=bo_e.ap()[None, :].to_broadcast((P, D)))
            if include_biases:
                bkv_bc = const_pool.tile([P, 2, D], f32, name="bkv_bc")
                nc.sync.dma_start(bkv_bc[:, 0, :], bk_e.ap()[None, :].to_broadcast((P, D)))
                nc.sync.dma_start(bkv_bc[:, 1, :], bv_e.ap()[None, :].to_broadcast((P, D)))
            id_sb = const_pool.tile([P, P], bf16, name="id_sb")
            make_identity(nc, id_sb[:])
            ones_sb = const_pool.tile([P, HD], bf16, name="ones_sb")
            nc.vector.memset(ones_sb[:], 1.0)

            # ---------------- X^T: cast to SBUF, XBAR-transpose per s-chunk ----------------
            # xT layout: [d_in(P), sc(16), dc(8), s_in(128)]
            xT, free_xT = tc.tile([P, NSC, NDC, P], bf16, name="xT")
            xstage, free_xstage = tc.tile([P, NSC, D], bf16, name="xstage")
            nc.gpsimd.dma_start(
                out=xstage[:], in_=X_e.ap().rearrange("(o p) n -> p o n", p=P)
            )
            for scg in range(4):
                nc.sync.dma_start_transpose(
                    xT[:, scg * 4 : (scg + 1) * 4, :, :],
                    xstage[:, scg * 4 : (scg + 1) * 4, :],
                )
            free_xstage()

            qT_lo, free_qT_lo = tc.tile([P, 4, SL], bf16, name="qT_lo")
            wq_bf, free_wq = tc.tile([P, NDC, D], bf16, name="wq_bf")
            kv, free_kv = tc.tile([P, NSC, H, 2, HD], bf16, name="kv")
            eT_scope = ExitStack()
            eT_pool = eT_scope.enter_context(tc.tile_pool(name="eT", bufs=5))
            part_pool = eT_scope.enter_context(tc.tile_pool(name="part", bufs=3))
            wk_bf, free_wk = tc.tile([P, NDC, D], bf16, name="wk_bf")
            wv_bf, free_wv = tc.tile([P, NDC, D], bf16, name="wv_bf")
            nc.gpsimd.dma_start(
                out=wk_bf[:], in_=Wk_e.ap().rearrange("(o p) n -> p o n", p=P)
            )
            nc.gpsimd.dma_start(
                out=wv_bf[:], in_=Wv_e.ap().rearrange("(o p) n -> p o n", p=P)
            )
            nc.gpsimd.dma_start(
                out=wq_bf[:], in_=Wq_e.ap().rearrange("(o p) n -> p o n", p=P)
            )

            # ---------------- E: cast to DRAM scratch, per-head XBAR-transpose ----------------
            for h in range(H):
                nc.gpsimd.dma_start(out=ebf_d[h], in_=E_e[h])
            # eT layout per head: [s_in(P), so(16), k(256)]
            eT_tiles = {}

            def stage_eT(h):
                eT = eT_pool.tile([P, NSC, LK], bf16, name="eT")
                nc.sync.dma_start_transpose(eT[:], ebf_d[h])
                eT_tiles[h] = eT

            for h in range(5):
                stage_eT(h)

            with (
                tc.tile_pool(name="ps_kvq", bufs=4, space="PSUM") as ps_kvq,
                tc.tile_pool(name="ps_part", bufs=2, space="PSUM") as ps_part,
            ):
                # ---------------- K/V natural [s, dh] (masked, bf16) ----------------
                for sc in range(NSC):
                    for t, w_bf in ((0, wk_bf), (1, wv_bf)):
                        for half in range(2):
                            ps = ps_kvq.tile([P, 512], f32, name="ps_kv", tag="mm512")
                            for dc in range(NDC):
                                nc.tensor.matmul(
                                    ps[:],
                                    xT[:, sc, dc, :],
                                    w_bf[:, dc, half * 512 : (half + 1) * 512],
                                    start=(dc == 0),
                                    stop=(dc == NDC - 1),
                                )
                            if include_biases:
                                nc.vector.tensor_tensor(
                                    out=ps[:],
                                    in0=ps[:],
                                    in1=bkv_bc[:, t, half * 512 : (half + 1) * 512],
                                    op=mybir.AluOpType.add,
                                )
                            nc.vector.tensor_scalar(
                                out=kv[:, sc, half * 8 : (half + 1) * 8, t, :],
                                in0=ps[:],
                                scalar1=m_sb[:, sc : sc + 1],
                                scalar2=None,
                                op0=mybir.AluOpType.mult,
                            )

                def q_chunk(mc, dst, dj):
                    for sn in range(NSN):
                        ps = ps_kvq.tile([P, 512], f32, name="psq", tag="mm512")
                        for dc in range(NDC):
                            nc.tensor.matmul(
                                ps[:],
                                wq_bf[:, dc, mc * P : (mc + 1) * P],
                                xT[:, sn * 4 : (sn + 1) * 4, dc, :],
                                start=(dc == 0),
                                stop=(dc == NDC - 1),
                            )
                        nc.vector.tensor_scalar(
                            out=dst[:, mc - dj, sn * 512 : (sn + 1) * 512],
                            in0=ps[:],
                            scalar1=bq_sb[:, mc : mc + 1],
                            scalar2=None,
                            op0=mybir.AluOpType.add,
                        )

                # ---------------- partial [KpT; VpT] per head (Q low chunks woven in) ----------------
                for h in range(H):
                    if h in (5, 8, 11, 14):
                        q_chunk((h - 5) // 3, qT_lo, 0)
                    eT = eT_tiles.pop(h)
                    kp_ps = ps_part.tile([P, LK], f32, name="kp_ps")
                    for so in range(NSC):
                        nc.tensor.matmul(
                            kp_ps[:],
                            kv[:, so, h, :, :],
                            eT[:, so, :],
                            start=(so == 0),
                            stop=(so == NSC - 1),
                        )
                    kp_sb = part_pool.tile([P, LK], bf16, name="kp_sb")
                    nc.vector.tensor_copy(kp_sb[:], kp_ps[:])
                    nc.gpsimd.dma_start(
                        out=cc_in[h].rearrange("(a b) -> a b", a=P),
                        in_=kp_sb[:],
                    )
                    if h + 5 < H:
                        stage_eT(h + 5)

                # ---------------- AllReduce over pairs (bf16, 1 MiB) ----------------
                nc.gpsimd.collective_compute(
                    "AllReduce",
                    mybir.AluOpType.add,
                    replica_groups=PAIRS,
                    ins=[cc_in[:].opt()],
                    outs=[cc_out[:].opt()],
                )

            free_wv()
            free_wk()
            eT_scope.close()
            free_kv()


            # ---------------- Q^T high half (covers the AllReduce) ----------------
            qT_hi, free_qT_hi = tc.tile([P, 4, SL], bf16, name="qT_hi")
            with tc.tile_pool(name="ps_q", bufs=4, space="PSUM") as ps_q:
                for mc in range(4, NDC):
                    for sn in range(NSN):
                        ps = ps_q.tile([P, 512], f32, name="psq2")
                        for dc in range(NDC):
                            nc.tensor.matmul(
                                ps[:],
                                wq_bf[:, dc, mc * P : (mc + 1) * P],
                                xT[:, sn * 4 : (sn + 1) * 4, dc, :],
                                start=(dc == 0),
                                stop=(dc == NDC - 1),
                            )
                        nc.vector.tensor_scalar(
                            out=qT_hi[:, mc - 4, sn * 512 : (sn + 1) * 512],
                            in0=ps[:],
                            scalar1=bq_sb[:, mc : mc + 1],
                            scalar2=None,
                            op0=mybir.AluOpType.add,
                        )

            # ---------------- read back reduced Kp^T / Vp ----------------
            kpT, free_kpT = tc.tile([P, H // 2, LK], bf16, name="kpT")
            vp_sb2, free_vp = tc.tile([P, H, 2, HD], bf16, name="vp_sb2")
            with (
                tc.tile_pool(name="vpT_pool", bufs=3) as vpT_pool,
                tc.tile_pool(name="ps_tp", bufs=3, space="PSUM") as ps_tp,
            ):
                for h in range(H):
                    par = (h % 2) * 64
                    nc.gpsimd.dma_start(
                        out=kpT[par : par + 64, h // 2, :],
                        in_=cc_out[h, 0 : 64 * LK].rearrange("(a b) -> a b", a=64),
                    )
                    vpT_sb = vpT_pool.tile([64, 2, P], bf16, name="vpT_sb")
                    nc.gpsimd.dma_start(
                        out=vpT_sb[:],
                        in_=cc_out[h, 64 * LK :].rearrange("(a b) -> a b", a=64),
                    )
                    for c in range(2):
                        tp_ps = ps_tp.tile([P, HD], bf16, name="tp_ps")
                        nc.tensor.transpose(
                            tp_ps[:], vpT_sb[:, c, :], id_sb[0:64, 0:64]
                        )
                        nc.vector.tensor_copy(vp_sb2[:, h, c, :], tp_ps[:])

            # ---------------- attention (sn outer) + inline output projection ----------------
            xoT, free_xoT = tc.tile([P, NDC, SL], bf16, name="xoT")
            wo_bf, free_wo = tc.tile([P, NDC, D], bf16, name="wo_bf")
            nc.gpsimd.dma_start(
                out=wo_bf[:], in_=Wo_e.ap().rearrange("(o p) n -> p o n", p=P)
            )
            with (
                tc.tile_pool(name="at_pool", bufs=3) as at_pool,
                tc.tile_pool(name="rbc_pool", bufs=2) as rbc_pool,
                tc.tile_pool(name="osb_pool", bufs=3) as osb_pool,
                tc.tile_pool(name="ps_dot", bufs=2, space="PSUM") as ps_dot,
                tc.tile_pool(name="ps_xoden", bufs=4, space="PSUM") as ps_xoden,
                tc.tile_pool(name="ps_out", bufs=2, space="PSUM") as ps_out,
            ):
                def attn_pair(sn, j):
                    # heads (2j, 2j+1): even parity on partitions 0-63, odd on 64-127
                    ssl = slice(sn * 512, (sn + 1) * 512)
                    ats = []
                    for par in (0, 64):
                        at = at_pool.tile([P, 2, 512], bf16, name="at")
                        for kc in range(2):
                            dps = ps_dot.tile([P, 512], f32, name="dps")
                            qsrc = qT_lo if j < 4 else qT_hi
                            nc.tensor.matmul(
                                dps[:],
                                kpT[par : par + 64, j, kc * P : (kc + 1) * P],
                                qsrc[par : par + 64, j % 4, ssl],
                                start=True,
                                stop=True,
                            )
                            nc.scalar.activation(
                                out=at[:, kc, :],
                                in_=dps[:],
                                func=mybir.ActivationFunctionType.Exp,
                                scale=0.125,
                            )
                        ats.append(at)
                    xo_ps = ps_xoden.tile([P, 512], f32, name="xo_ps", tag="xoden")
                    den_ps = ps_xoden.tile([P, 512], f32, name="den_ps", tag="xoden")
                    for kc in range(2):
                        for pi, par in ((0, 0), (1, 64)):
                            h = 2 * j + pi
                            nc.tensor.matmul(
                                xo_ps[par : par + 64, :],
                                vp_sb2[:, h, kc, :],
                                ats[pi][:, kc, :],
                                start=(kc == 0),
                                stop=(kc == 1),
                                skip_group_check=True,
                            )
                            nc.tensor.matmul(
                                den_ps[par : par + 64, :],
                                ones_sb[:],
                                ats[pi][:, kc, :],
                                start=(kc == 0),
                                stop=(kc == 1),
                                skip_group_check=True,
                            )
                    rbc = rbc_pool.tile([P, 512], f32, name="rbc")
                    nc.vector.reciprocal_approx_fast(out=rbc[:], in_=den_ps[:])
                    nc.vector.tensor_tensor(
                        out=xoT[:, j, ssl],
                        in0=xo_ps[:],
                        in1=rbc[:],
                        op=mybir.AluOpType.mult,
                    )

                for sn in range(NSN):
                    for j in range(H // 2):
                        attn_pair(sn, j)
                    # output projection for this sn group (all heads now done)
                    for si in range(4):
                        sc = sn * 4 + si
                        for half in range(2):
                            ps = ps_out.tile([P, 512], f32, name="ps_o")
                            for c in range(NDC):
                                nc.tensor.matmul(
                                    ps[:],
                                    xoT[:, c, sc * P : (sc + 1) * P],
                                    wo_bf[:, c, half * 512 : (half + 1) * 512],
                                    start=(c == 0),
                                    stop=(c == NDC - 1),
                                )
                            osb = osb_pool.tile([P, 512], f32, name="osb")
                            nc.vector.tensor_tensor(
                                out=osb[:],
                                in0=ps[:],
                                in1=bo_bc[:, half * 512 : (half + 1) * 512],
                                op=mybir.AluOpType.add,
                            )
                            nc.sync.dma_start(
                                out=out_e[sc * P : (sc + 1) * P, half * 512 : (half + 1) * 512],
                                in_=osb[:],
                            )
            if debug:
                dbg_kpT = nc.declare_dram_parameter("dbg_kpT", [P, H // 2, LK], f32, isOutput=True)
                dbg_vp = nc.declare_dram_parameter("dbg_vp", [P, H, 2, HD], f32, isOutput=True)
                dbg_qT = nc.declare_dram_parameter("dbg_qT", [P, NDC, SL], f32, isOutput=True)
                dbg_xoT = nc.declare_dram_parameter("dbg_xoT", [P, NDC, SL], f32, isOutput=True)
                nc.gpsimd.dma_start(out=dbg_kpT[:], in_=kpT[:])
                nc.gpsimd.dma_start(out=dbg_vp[:], in_=vp_sb2[:])
                nc.gpsimd.dma_start(out=dbg_qT[:], in_=qT[:])
                nc.gpsimd.dma_start(out=dbg_xoT[:], in_=xoT[:])
            free_wo()
            free_xoT()
            free_vp()
            free_kpT()
            free_qT_hi()
            free_wq()
            free_qT_lo()
            free_xT()

    nc.compile()
    return nc


_cache = {}


def _get_nc(include_biases: bool):
    if include_biases not in _cache:
        _cache[include_biases] = _build(include_biases)
    return _cache[include_biases]


def kernel(**inputs) -> np.ndarray:
    X = np.asarray(inputs["X"], np.float32)
    mask = np.asarray(inputs["mask"], np.float32)
    E = np.asarray(inputs["E"], np.float32)
    Ws = {k: np.asarray(inputs[k], np.float32) for k in ("Wq", "Wk", "Wv", "Wo")}
    bs = {k: np.asarray(inputs[k], np.float32) for k in ("bq", "bk", "bv", "bo")}

    include_biases = bool(np.any(bs["bk"]) or np.any(bs["bv"]))
    nc = _get_nc(include_biases)

    in_maps = []
    for c in range(8):
        b, half = c // 2, c % 2
        sl = slice(half * SL, (half + 1) * SL)
        in_maps.append(
            {
                "X": np.ascontiguousarray(X[b, sl, :]),
                "mask": np.ascontiguousarray(mask[b, sl]),
                "Wq": Ws["Wq"], "bq": bs["bq"],
                "Wk": Ws["Wk"], "bk": bs["bk"],
                "Wv": Ws["Wv"], "bv": bs["bv"],
                "E": np.ascontiguousarray(E[:, :, sl]),
                "Wo": Ws["Wo"], "bo": bs["bo"],
            }
        )

    res = bass_utils.run_bass_kernel_spmd(nc, in_maps, core_ids=list(range(8)))
    out = np.empty((B, S, D), np.float32)
    for c in range(8):
        b, half = c // 2, c % 2
        out[b, half * SL : (half + 1) * SL, :] = res.results[c]["out"]
    return out



# revision 7
# speedup vs baseline: 1.5468x; 1.5468x over previous
"""Trainium2 Bass kernel for Linformer-style sparse attention.

Problem shapes (hardcoded): B=4, S=4096, D=1024, H=16, HD=64, LK=256.

Sharding (8 cores): core c -> (batch b = c//2, sequence half = c%2).
Each core:
  - computes Q/K/V for its 2048 rows (all heads),
  - computes partial [Kp^T; Vp^T] = (K|V)^T @ E^T over its rows,
  - pair AllReduce ([0,1],[2,3],[4,5],[6,7]) completes Kp/Vp (bf16, split
    into two 0.5 MiB halves so each hides under compute),
  - attention (softmax over LK=256) + output projection for its own rows,
  - writes its [2048, 1024] slice of the output directly (no final collective).

All inputs are pre-transposed and pre-cast to bf16 on the host in exact
SBUF layouts, so the device does no transposes or cast round-trips:
  XT [128, 8, 2048]       X^T with d=dc*128+p
  EA/EB [16, 128, 8, 256] E^T per seq-chunk for head groups 0-7 / 8-15
  WK/WV/WQ/WO [128, 8, 1024] with d_in = o*128+p; WQ pre-scaled by 1/sqrt(HD)
The K/V projection is fused with the Kp/Vp partial accumulation (PSUM
accumulators across the 16 seq-chunks), E^T streams in 0.5 MiB chunks,
and the Q projection covers the collectives.
"""

import sys

sys.path.insert(0, "/opt/trn_rl_repo")

from contextlib import ExitStack

import numpy as np
import ml_dtypes

from concourse import bacc, bass_utils, mybir, tile
from concourse.masks import make_identity

B, S, D = 4, 4096, 1024
H, HD, LK = 16, 64, 256
SL = S // 2            # local sequence rows per core
P = 128
NSC = SL // P          # 16 s-chunks of 128
NDC = D // P           # 8 d-chunks of 128
NSN = SL // 512        # 4 s-blocks of 512
f32 = mybir.dt.float32
bf16 = mybir.dt.bfloat16
PAIRS = [[0, 1], [2, 3], [4, 5], [6, 7]]
BF16 = ml_dtypes.bfloat16


def _build(include_biases: bool, debug: bool = False):
    nc = bacc.Bacc("TRN2", target_bir_lowering=False, num_devices=8)

    XT_e = nc.declare_dram_parameter("XT", [P, NDC, SL], bf16, isOutput=False)
    EA_e = nc.declare_dram_parameter("EA", [NSC, P, 8, LK], bf16, isOutput=False)
    EB_e = nc.declare_dram_parameter("EB", [NSC, P, 8, LK], bf16, isOutput=False)
    WK_e = nc.declare_dram_parameter("WK", [P, NDC, D], bf16, isOutput=False)
    WV_e = nc.declare_dram_parameter("WV", [P, NDC, D], bf16, isOutput=False)
    WQ_e = nc.declare_dram_parameter("WQ", [P, NDC, D], bf16, isOutput=False)
    WO_e = nc.declare_dram_parameter("WO", [P, NDC, D], bf16, isOutput=False)
    BQ_e = nc.declare_dram_parameter("BQ", [P, NDC], f32, isOutput=False)
    MS_e = nc.declare_dram_parameter("MS", [P, NSC], f32, isOutput=False)
    BO_e = nc.declare_dram_parameter("BO", [D], f32, isOutput=False)
    if include_biases:
        BKV_e = nc.declare_dram_parameter("BKV", [2, D], f32, isOutput=False)
    out_e = nc.declare_dram_parameter("out", [SL, D], f32, isOutput=True)
    if debug:
        dbg_cca = nc.declare_dram_parameter("dbg_cca", [8, P, LK], f32, isOutput=True)
        dbg_ccb = nc.declare_dram_parameter("dbg_ccb", [8, P, LK], f32, isOutput=True)
        dbg_qT = nc.declare_dram_parameter("dbg_qT", [P, NDC, SL], f32, isOutput=True)
        dbg_kpT = nc.declare_dram_parameter("dbg_kpT", [P, H // 2, LK], f32, isOutput=True)
        dbg_vp = nc.declare_dram_parameter("dbg_vp", [P, H, 2, HD], f32, isOutput=True)
        dbg_xoT = nc.declare_dram_parameter("dbg_xoT", [P, NDC, SL], f32, isOutput=True)

    # AllReduce bounce (bf16): per head [KpT ; VpT] stacked [128, 256]
    cc_in_a = nc.dram_tensor("cc_in_a", [8, P, LK], bf16, kind="Internal")
    cc_out_a = nc.dram_tensor("cc_out_a", [8, P, LK], bf16, kind="Internal")
    cc_in_b = nc.dram_tensor("cc_in_b", [8, P, LK], bf16, kind="Internal")
    cc_out_b = nc.dram_tensor("cc_out_b", [8, P, LK], bf16, kind="Internal")

    with tile.TileContext(nc) as tc:
        ctx = ExitStack()
        with ctx:
            const_pool = ctx.enter_context(tc.tile_pool(name="consts", bufs=1))

            # ---------------- constants ----------------
            m_sb = const_pool.tile([P, NSC], f32, name="m_sb")
            nc.sync.dma_start(m_sb[:], MS_e.ap())
            bq_sb = const_pool.tile([P, NDC], f32, name="bq_sb")
            nc.sync.dma_start(bq_sb[:], BQ_e.ap())
            bo_bc = const_pool.tile([P, D], f32, name="bo_bc")
            nc.gpsimd.dma_start(out=bo_bc[:], in_=BO_e.ap()[None, :].to_broadcast((P, D)))
            if include_biases:
                bkv_bc = const_pool.tile([P, 2, D], f32, name="bkv_bc")
                nc.gpsimd.dma_start(
                    out=bkv_bc[:, 0, :], in_=BKV_e.ap()[0][None, :].to_broadcast((P, D))
                )
                nc.gpsimd.dma_start(
                    out=bkv_bc[:, 1, :], in_=BKV_e.ap()[1][None, :].to_broadcast((P, D))
                )
            id_sb = const_pool.tile([P, P], bf16, name="id_sb")
            make_identity(nc, id_sb[:])
            ones_sb = const_pool.tile([P, HD], bf16, name="ones_sb")
            nc.vector.memset(ones_sb[:], 1.0)

            # ---------------- persistent tiles (stack order matters: LIFO frees) ---
            kpT, free_kpT = tc.tile([P, H // 2, LK], bf16, name="kpT")
            vp_sb2, free_vp = tc.tile([P, H, 2, HD], bf16, name="vp_sb2")
            qT, free_qT = tc.tile([P, NDC, SL], bf16, name="qT")
            wq_sb, free_wq = tc.tile([P, NDC, D], bf16, name="wq_sb")
            kvB, free_kvB = tc.tile([P, NSC, 8, 2, HD], bf16, name="kvB")
            xt_sb, free_xt = tc.tile([P, NDC, SL], bf16, name="xt_sb")
            wk_sb, free_wk = tc.tile([P, NDC, D], bf16, name="wk_sb")
            wv_sb, free_wv = tc.tile([P, NDC, D], bf16, name="wv_sb")
            kp_scope = ExitStack()
            kp_pool = kp_scope.enter_context(
                tc.tile_pool(name="kp_sb", bufs=3, side="right")
            )

            nc.sync.dma_start(xt_sb[:], XT_e.ap())
            nc.sync.dma_start(wk_sb[:], WK_e.ap())
            nc.sync.dma_start(wv_sb[:], WV_e.ap())
            nc.sync.dma_start(wq_sb[:], WQ_e.ap())

            eB_scope = ExitStack()
            eB_pool = eB_scope.enter_context(
                tc.tile_pool(name="eB", bufs=4, side="right")
            )

            # ---------------- pass A: K/V projection fused with partials h0-7 ----
            passA_scope = ExitStack()
            kvs_pool = passA_scope.enter_context(
                tc.tile_pool(name="kvs", bufs=3, side="right")
            )
            eA_pool = passA_scope.enter_context(
                tc.tile_pool(name="eA", bufs=4, side="right")
            )
            psA_scope = ExitStack()
            kv_ps = psA_scope.enter_context(
                tc.tile_pool(name="kv_ps", bufs=4, space="PSUM")
            )
            kpA_ps = psA_scope.enter_context(
                tc.tile_pool(name="kpA_ps", bufs=1, space="PSUM")
            )
            kpA = [kpA_ps.tile([P, 2, LK], f32, name=f"kpA{i}") for i in range(4)]

            def kv_chunk(sc, kvs, kv_b_dst):
                # K/V for s-chunk sc, all heads; heads 0-7 -> kvs, 8-15 -> kvB
                for t, w_sb in ((0, wk_sb), (1, wv_sb)):
                    for half in range(2):
                        ps = kv_ps.tile([P, 512], f32, name="ps_kv", tag="mm512")
                        for dc in range(NDC):
                            nc.tensor.matmul(
                                ps[:],
                                xt_sb[:, dc, sc * P : (sc + 1) * P],
                                w_sb[:, dc, half * 512 : (half + 1) * 512],
                                start=(dc == 0),
                                stop=(dc == NDC - 1),
                            )
                        if include_biases:
                            nc.vector.tensor_tensor(
                                out=ps[:],
                                in0=ps[:],
                                in1=bkv_bc[:, t, half * 512 : (half + 1) * 512],
                                op=mybir.AluOpType.add,
                            )
                        dst = kvs[:, :, t, :] if half == 0 else kv_b_dst[:, :, t, :]
                        nc.vector.tensor_scalar(
                            out=dst,
                            in0=ps[:],
                            scalar1=m_sb[:, sc : sc + 1],
                            scalar2=None,
                            op0=mybir.AluOpType.mult,
                        )

            for sc in range(NSC):
                eA = eA_pool.tile([P, 8, LK], bf16, name="eA")
                nc.sync.dma_start(eA[:], EA_e[sc])
                kvs = kvs_pool.tile([P, 8, 2, HD], bf16, name="kvs")
                kv_chunk(sc, kvs, kvB[:, sc])
                for hp in range(4):
                    for i in range(2):
                        h = 2 * hp + i
                        nc.tensor.matmul(
                            kpA[hp][:, i, :],
                            kvs[:, h, :, :],
                            eA[:, h, :],
                            start=(sc == 0 and i == 0),
                            stop=(sc == NSC - 1 and i == 1),
                            skip_group_check=True,
                        )

            # ship partials A, kick first AllReduce
            for hp in range(4):
                kp_sb = kp_pool.tile([P, 2, LK], bf16, name="kp_sb")
                nc.vector.tensor_copy(kp_sb[:], kpA[hp][:])
                for i in range(2):
                    nc.gpsimd.dma_start(out=cc_in_a[2 * hp + i], in_=kp_sb[:, i, :])
            psA_scope.close()
            passA_scope.close()
            free_wv()
            free_wk()
            nc.gpsimd.collective_compute(
                "AllReduce",
                mybir.AluOpType.add,
                replica_groups=PAIRS,
                ins=[cc_in_a[:].opt()],
                outs=[cc_out_a[:].opt()],
            )

            # ---------------- Q projection groups (shared by pass B & tail) -----
            q_groups = [(mc, sn) for mc in range(NDC) for sn in range(NSN)]

            def q_group(ps_pool, mc, sn):
                ps = ps_pool.tile([P, 512], f32, name="psq")
                for dc in range(NDC):
                    nc.tensor.matmul(
                        ps[:],
                        wq_sb[:, dc, mc * P : (mc + 1) * P],
                        xt_sb[:, dc, sn * 512 : (sn + 1) * 512],
                        start=(dc == 0),
                        stop=(dc == NDC - 1),
                    )
                nc.vector.tensor_scalar(
                    out=qT[:, mc, sn * 512 : (sn + 1) * 512],
                    in0=ps[:],
                    scalar1=bq_sb[:, mc : mc + 1],
                    scalar2=None,
                    op0=mybir.AluOpType.add,
                )

            # ---------------- pass B: partials h8-15 + half the Q groups --------
            psB_scope = ExitStack()
            kpB_ps = psB_scope.enter_context(
                tc.tile_pool(name="kpB_ps", bufs=1, space="PSUM")
            )
            qB_ps = psB_scope.enter_context(
                tc.tile_pool(name="qB_ps", bufs=3, space="PSUM")
            )
            kpB = [kpB_ps.tile([P, 2, LK], f32, name=f"kpB{i}") for i in range(4)]
            for sc in range(NSC):
                eB = eB_pool.tile([P, 8, LK], bf16, name="eB")
                nc.sync.dma_start(eB[:], EB_e[sc])
                for hp in range(4):
                    for i in range(2):
                        h = 2 * hp + i
                        nc.tensor.matmul(
                            kpB[hp][:, i, :],
                            kvB[:, sc, h, :, :],
                            eB[:, h, :],
                            start=(sc == 0 and i == 0),
                            stop=(sc == NSC - 1 and i == 1),
                            skip_group_check=True,
                        )
                q_group(qB_ps, *q_groups[sc])

            for hp in range(4):
                kp_sb = kp_pool.tile([P, 2, LK], bf16, name="kp_sb")
                nc.vector.tensor_copy(kp_sb[:], kpB[hp][:])
                for i in range(2):
                    nc.gpsimd.dma_start(out=cc_in_b[2 * hp + i], in_=kp_sb[:, i, :])
            psB_scope.close()
            eB_scope.close()
            kp_scope.close()
            nc.gpsimd.collective_compute(
                "AllReduce",
                mybir.AluOpType.add,
                replica_groups=PAIRS,
                ins=[cc_in_b[:].opt()],
                outs=[cc_out_b[:].opt()],
            )

            # ---------------- readbacks + remaining Q (covers AllReduce B) ------
            rb_scope = ExitStack()
            vpT_pool = rb_scope.enter_context(
                tc.tile_pool(name="vpT", bufs=3, side="right")
            )
            tp_ps_pool = rb_scope.enter_context(
                tc.tile_pool(name="tp_ps", bufs=2, space="PSUM")
            )
            qT_ps_pool = rb_scope.enter_context(
                tc.tile_pool(name="qT_ps", bufs=3, space="PSUM")
            )

            def readback(cc_out, hbase):
                for hl in range(8):
                    h = hbase + hl
                    par = (h % 2) * 64
                    nc.gpsimd.dma_start(
                        out=kpT[par : par + 64, h // 2, :], in_=cc_out[hl, 0:64, :]
                    )
                    vpT_sb = vpT_pool.tile([64, 2, P], bf16, name="vpT_sb")
                    nc.gpsimd.dma_start(out=vpT_sb[:], in_=cc_out[hl, 64:128, :])
                    for c in range(2):
                        tp_ps = tp_ps_pool.tile([P, HD], bf16, name="tp_ps")
                        nc.tensor.transpose(
                            tp_ps[:], vpT_sb[:, c, :], id_sb[0:64, 0:64]
                        )
                        nc.vector.tensor_copy(vp_sb2[:, h, c, :], tp_ps[:])

            readback(cc_out_a, 0)
            for mc, sn in q_groups[NSC:]:
                q_group(qT_ps_pool, mc, sn)
            readback(cc_out_b, 8)
            rb_scope.close()
            free_xt()
            free_kvB()

            # ---------------- attention + inline output projection --------------
            wo_sb, free_wo = tc.tile([P, NDC, D], bf16, name="wo_sb")
            nc.sync.dma_start(wo_sb[:], WO_e.ap())
            xoT, free_xoT = tc.tile([P, NDC, SL], bf16, name="xoT")

            with (
                tc.tile_pool(name="at_pool", bufs=3, side="right") as at_pool,
                tc.tile_pool(name="rbc_pool", bufs=2, side="right") as rbc_pool,
                tc.tile_pool(name="osb_pool", bufs=3, side="right") as osb_pool,
                tc.tile_pool(name="ps_dot", bufs=4, space="PSUM") as ps_dot,
                tc.tile_pool(name="ps_xoden", bufs=2, space="PSUM") as ps_xoden,
                tc.tile_pool(name="ps_out", bufs=2, space="PSUM") as ps_out,
            ):
                def attn_dot(sn, j):
                    # heads (2j, 2j+1): even head on partitions 0-63, odd on 64-127
                    ssl = slice(sn * 512, (sn + 1) * 512)
                    ats = []
                    dps = {}
                    for kc in range(2):
                        for pi, par in ((0, 0), (1, 64)):
                            d = ps_dot.tile([P, 512], f32, name="dps")
                            nc.tensor.matmul(
                                d[:],
                                kpT[par : par + 64, j, kc * P : (kc + 1) * P],
                                qT[par : par + 64, j, ssl],
                                start=True,
                                stop=True,
                            )
                            dps[(kc, pi)] = d
                    for pi in range(2):
                        at = at_pool.tile([P, 2, 512], bf16, name="at")
                        for kc in range(2):
                            nc.scalar.activation(
                                out=at[:, kc, :],
                                in_=dps[(kc, pi)][:],
                                func=mybir.ActivationFunctionType.Exp,
                            )
                        ats.append(at)
                    return ats

                def attn_pv(sn, j, ats):
                    ssl = slice(sn * 512, (sn + 1) * 512)
                    xo_ps = ps_xoden.tile([P, 512], f32, name="xo_ps", tag="xoden")
                    den_ps = ps_xoden.tile([P, 512], f32, name="den_ps", tag="xoden")
                    for kc in range(2):
                        for pi, par in ((0, 0), (1, 64)):
                            h = 2 * j + pi
                            nc.tensor.matmul(
                                xo_ps[par : par + 64, :],
                                vp_sb2[:, h, kc, :],
                                ats[pi][:, kc, :],
                                start=(kc == 0),
                                stop=(kc == 1),
                                skip_group_check=True,
                            )
                            nc.tensor.matmul(
                                den_ps[par : par + 64, :],
                                ones_sb[:],
                                ats[pi][:, kc, :],
                                start=(kc == 0),
                                stop=(kc == 1),
                                skip_group_check=True,
                            )
                    rbc = rbc_pool.tile([P, 512], f32, name="rbc")
                    nc.vector.reciprocal_approx_fast(out=rbc[:], in_=den_ps[:])
                    nc.vector.tensor_tensor(
                        out=xoT[:, j, ssl],
                        in0=xo_ps[:],
                        in1=rbc[:],
                        op=mybir.AluOpType.mult,
                    )

                def outproj(sn):
                    for si in range(4):
                        sc = sn * 4 + si
                        for half in range(2):
                            ps = ps_out.tile([P, 512], f32, name="ps_o")
                            for c in range(NDC):
                                nc.tensor.matmul(
                                    ps[:],
                                    xoT[:, c, sc * P : (sc + 1) * P],
                                    wo_sb[:, c, half * 512 : (half + 1) * 512],
                                    start=(c == 0),
                                    stop=(c == NDC - 1),
                                )
                            osb = osb_pool.tile([P, 512], f32, name="osb")
                            nc.vector.tensor_tensor(
                                out=osb[:],
                                in0=ps[:],
                                in1=bo_bc[:, half * 512 : (half + 1) * 512],
                                op=mybir.AluOpType.add,
                            )
                            nc.sync.dma_start(
                                out=out_e[
                                    sc * P : (sc + 1) * P, half * 512 : (half + 1) * 512
                                ],
                                in_=osb[:],
                            )

                # software-pipelined by one iteration: dot(i+1) is emitted before
                # pv(i) so the PE FIFO never stalls on the exp of the current tile
                iters = [(sn, j) for sn in range(NSN) for j in range(H // 2)]
                pending = None
                for sn, j in iters:
                    ats = attn_dot(sn, j)
                    if pending is not None:
                        psn, pj, pats = pending
                        attn_pv(psn, pj, pats)
                        if pj == H // 2 - 1:
                            outproj(psn)
                    pending = (sn, j, ats)
                psn, pj, pats = pending
                attn_pv(psn, pj, pats)
                outproj(psn)

            if debug:
                nc.gpsimd.dma_start(out=dbg_cca[:], in_=cc_in_a[:])
                nc.gpsimd.dma_start(out=dbg_ccb[:], in_=cc_in_b[:])
                nc.gpsimd.dma_start(out=dbg_qT[:], in_=qT[:])
                nc.gpsimd.dma_start(out=dbg_kpT[:], in_=kpT[:])
                nc.gpsimd.dma_start(out=dbg_vp[:], in_=vp_sb2[:])
                nc.gpsimd.dma_start(out=dbg_xoT[:], in_=xoT[:])
            free_xoT()
            free_wo()
            free_wq()
            free_qT()
            free_vp()
            free_kpT()

    nc.compile()
    return nc


_cache = {}


def _get_nc(include_biases: bool, debug: bool = False):
    key = (include_biases, debug)
    if key not in _cache:
        _cache[key] = _build(include_biases, debug)
    return _cache[key]


def prepare_in_maps(inputs):
    X = np.asarray(inputs["X"], np.float32)
    mask = np.asarray(inputs["mask"], np.float32)
    E = np.asarray(inputs["E"], np.float32)
    Ws = {k: np.asarray(inputs[k], np.float32) for k in ("Wq", "Wk", "Wv", "Wo")}
    bs = {k: np.asarray(inputs[k], np.float32) for k in ("bq", "bk", "bv", "bo")}

    include_biases = bool(np.any(bs["bk"]) or np.any(bs["bv"]))

    def wprep(w):
        return np.ascontiguousarray(
            w.reshape(NDC, P, D).transpose(1, 0, 2)
        ).astype(BF16)

    WK = wprep(Ws["Wk"])
    WV = wprep(Ws["Wv"])
    WQ = wprep(Ws["Wq"] * 0.125)
    WO = wprep(Ws["Wo"])
    BQ = np.ascontiguousarray((bs["bq"] * 0.125).reshape(NDC, P).T)
    BO = bs["bo"]
    BKV = np.stack([bs["bk"], bs["bv"]])

    # E^T once: [S, H, LK]
    ET = np.ascontiguousarray(E.transpose(2, 0, 1))
    E_half = {}
    for half in range(2):
        sl = slice(half * SL, (half + 1) * SL)
        Eh = ET[sl].astype(BF16)  # [SL, H, LK]
        EA = np.ascontiguousarray(Eh[:, 0:8, :]).reshape(NSC, P, 8, LK)
        EB = np.ascontiguousarray(Eh[:, 8:16, :]).reshape(NSC, P, 8, LK)
        E_half[half] = (EA, EB)

    in_maps = []
    for c in range(8):
        b, half = c // 2, c % 2
        sl = slice(half * SL, (half + 1) * SL)
        XT = np.ascontiguousarray(
            X[b, sl, :].T.reshape(NDC, P, SL).transpose(1, 0, 2)
        ).astype(BF16)
        MS = np.ascontiguousarray(mask[b, sl].reshape(NSC, P).T)
        EA, EB = E_half[half]
        m = {
            "XT": XT, "EA": EA, "EB": EB,
            "WK": WK, "WV": WV, "WQ": WQ, "WO": WO,
            "BQ": BQ, "MS": MS, "BO": BO,
        }
        if include_biases:
            m["BKV"] = BKV
        in_maps.append(m)
    return include_biases, in_maps


def kernel(**inputs) -> np.ndarray:
    include_biases, in_maps = prepare_in_maps(inputs)
    nc = _get_nc(include_biases)
    res = bass_utils.run_bass_kernel_spmd(nc, in_maps, core_ids=list(range(8)))
    out = np.empty((B, S, D), np.float32)
    for c in range(8):
        b, half = c // 2, c % 2
        out[b, half * SL : (half + 1) * SL, :] = res.results[c]["out"]
    return out


# revision 9
# speedup vs baseline: 1.5659x; 1.0124x over previous
"""Trainium2 Bass kernel for Linformer-style sparse attention.

Problem shapes (hardcoded): B=4, S=4096, D=1024, H=16, HD=64, LK=256.

Sharding (8 cores): core c -> (batch b = c//2, sequence half = c%2).
Each core:
  - computes Q/K/V for its 2048 rows (all heads),
  - computes partial [Kp^T; Vp^T] = (K|V)^T @ E^T over its rows,
  - pair AllReduce ([0,1],[2,3],[4,5],[6,7]) completes Kp/Vp,
  - attention (softmax over LK=256) + output projection for its own rows,
  - writes its [2048, 1024] slice of the output directly (no final collective).

All inputs are pre-transposed and pre-cast to bf16 on the host in exact
SBUF layouts, so the device does no transposes or cast round-trips:
  XT [128, 8, 2048]       X^T with d=dc*128+p
  EA/EB [16, 128, 8, 256] E^T per seq-chunk for head groups 0-7 / 8-15
  WK/WV/WQ/WO [128, 8, 1024] with d_in = o*128+p; WQ pre-scaled by 1/sqrt(HD)

Schedule: the K/V projection runs in two per-head-group passes, each fused
with its Kp/Vp partial accumulation (PSUM accumulators across the 16
seq-chunks, E^T streaming in 0.5 MiB chunks).  AllReduce A kicks at ~50%
of the KV work and hides under pass B; AllReduce B hides under the Q
projection.  Readback DMAs ride the ACT HWDGE ring (the GpSimd SWDGE
desc-gen at ~0.6us each would serialize).  The attention loop is
software-pipelined one iteration ahead so the PE FIFO never waits on exp.
"""

import sys

sys.path.insert(0, "/opt/trn_rl_repo")

from contextlib import ExitStack

import numpy as np
import ml_dtypes

from concourse import bacc, bass_utils, mybir, tile
from concourse.masks import make_identity

B, S, D = 4, 4096, 1024
H, HD, LK = 16, 64, 256
SL = S // 2            # local sequence rows per core
P = 128
NSC = SL // P          # 16 s-chunks of 128
NDC = D // P           # 8 d-chunks of 128
NSN = SL // 512        # 4 s-blocks of 512
f32 = mybir.dt.float32
bf16 = mybir.dt.bfloat16
PAIRS = [[0, 1], [2, 3], [4, 5], [6, 7]]
BF16 = ml_dtypes.bfloat16


def _build(include_biases: bool, debug: bool = False):
    nc = bacc.Bacc("TRN2", target_bir_lowering=False, num_devices=8)

    XT_e = nc.declare_dram_parameter("XT", [P, NDC, SL], bf16, isOutput=False)
    EA_e = nc.declare_dram_parameter("EA", [NSC, P, 8, LK], bf16, isOutput=False)
    EB_e = nc.declare_dram_parameter("EB", [NSC, P, 8, LK], bf16, isOutput=False)
    WK_e = nc.declare_dram_parameter("WK", [P, NDC, D], bf16, isOutput=False)
    WV_e = nc.declare_dram_parameter("WV", [P, NDC, D], bf16, isOutput=False)
    WQ_e = nc.declare_dram_parameter("WQ", [P, NDC, D], bf16, isOutput=False)
    WO_e = nc.declare_dram_parameter("WO", [P, NDC, D], bf16, isOutput=False)
    BQ_e = nc.declare_dram_parameter("BQ", [P, NDC], f32, isOutput=False)
    MS_e = nc.declare_dram_parameter("MS", [P, NSC], f32, isOutput=False)
    BO_e = nc.declare_dram_parameter("BO", [D], f32, isOutput=False)
    if include_biases:
        BKV_e = nc.declare_dram_parameter("BKV", [2, D], f32, isOutput=False)
    out_e = nc.declare_dram_parameter("out", [SL, D], f32, isOutput=True)
    if debug:
        dbg_cca = nc.declare_dram_parameter("dbg_cca", [8, P, LK], f32, isOutput=True)
        dbg_ccb = nc.declare_dram_parameter("dbg_ccb", [8, P, LK], f32, isOutput=True)
        dbg_qT = nc.declare_dram_parameter("dbg_qT", [P, NDC, SL], f32, isOutput=True)
        dbg_kpT = nc.declare_dram_parameter("dbg_kpT", [P, H // 2, LK], f32, isOutput=True)
        dbg_vp = nc.declare_dram_parameter("dbg_vp", [P, H, 2, HD], f32, isOutput=True)
        dbg_xoT = nc.declare_dram_parameter("dbg_xoT", [P, NDC, SL], f32, isOutput=True)

    # AllReduce bounce (bf16): per head [KpT ; VpT] stacked [128, 256]
    cc_in_a = nc.dram_tensor("cc_in_a", [8, P, LK], bf16, kind="Internal")
    cc_out_a = nc.dram_tensor("cc_out_a", [8, P, LK], bf16, kind="Internal")
    cc_in_b = nc.dram_tensor("cc_in_b", [8, P, LK], bf16, kind="Internal")
    cc_out_b = nc.dram_tensor("cc_out_b", [8, P, LK], bf16, kind="Internal")

    with tile.TileContext(nc) as tc:
        ctx = ExitStack()
        with ctx:
            const_pool = ctx.enter_context(tc.tile_pool(name="consts", bufs=1))

            # ---------------- constants ----------------
            m_sb = const_pool.tile([P, NSC], f32, name="m_sb")
            nc.scalar.dma_start(m_sb[:], MS_e.ap())
            bq_sb = const_pool.tile([P, NDC], f32, name="bq_sb")
            nc.scalar.dma_start(bq_sb[:], BQ_e.ap())
            bo_bc = const_pool.tile([P, D], f32, name="bo_bc")
            nc.gpsimd.dma_start(out=bo_bc[:], in_=BO_e.ap()[None, :].to_broadcast((P, D)))
            if include_biases:
                bkv_bc = const_pool.tile([P, 2, D], f32, name="bkv_bc")
                nc.gpsimd.dma_start(
                    out=bkv_bc[:, 0, :], in_=BKV_e.ap()[0][None, :].to_broadcast((P, D))
                )
                nc.gpsimd.dma_start(
                    out=bkv_bc[:, 1, :], in_=BKV_e.ap()[1][None, :].to_broadcast((P, D))
                )
            id_sb = const_pool.tile([P, P], bf16, name="id_sb")
            make_identity(nc, id_sb[:])
            ones_sb = const_pool.tile([P, HD], bf16, name="ones_sb")
            nc.vector.memset(ones_sb[:], 1.0)

            # ------------- persistent tiles (left stack; frees must be LIFO) ----
            kpT, free_kpT = tc.tile([P, H // 2, LK], bf16, name="kpT")
            vp_sb2, free_vp = tc.tile([P, H, 2, HD], bf16, name="vp_sb2")
            qT, free_qT = tc.tile([P, NDC, SL], bf16, name="qT")
            wq_sb, free_wq = tc.tile([P, NDC, D], bf16, name="wq_sb")
            xt_sb, free_xt = tc.tile([P, NDC, SL], bf16, name="xt_sb")
            wk_sb, free_wk = tc.tile([P, NDC, D], bf16, name="wk_sb")
            wv_sb, free_wv = tc.tile([P, NDC, D], bf16, name="wv_sb")

            # prologue loads, ordered so the first KV matmuls unblock early
            nc.sync.dma_start(wk_sb[:], WK_e.ap())
            for q in range(4):
                nc.sync.dma_start(
                    xt_sb[:, :, q * 512 : (q + 1) * 512],
                    XT_e.ap()[:, :, q * 512 : (q + 1) * 512],
                )
            nc.sync.dma_start(wv_sb[:], WV_e.ap())

            # transient pools (right stack)
            kp_scope = ExitStack()
            kp_pool = kp_scope.enter_context(
                tc.tile_pool(name="kp_sb", bufs=4, side="right")
            )
            e_scope = ExitStack()
            e_pool = e_scope.enter_context(
                tc.tile_pool(name="e_pool", bufs=6, side="right")
            )
            kvs_scope = ExitStack()
            kvs_pool = kvs_scope.enter_context(
                tc.tile_pool(name="kvs", bufs=3, side="right")
            )

            ps_scope = ExitStack()
            kv_ps = ps_scope.enter_context(
                tc.tile_pool(name="kv_ps", bufs=4, space="PSUM")
            )

            def kv_pass(E_param, wcol, kp_acc, extra=None):
                # K/V projection for one head group (8 heads = weight columns
                # [wcol, wcol+512)) fused with the Kp/Vp partial accumulation.
                for sc in range(NSC):
                    eT = e_pool.tile([P, 8, LK], bf16, name="eT")
                    nc.sync.dma_start(eT[:], E_param[sc])
                    if extra is not None and sc in extra:
                        extra[sc]()
                    kvs = kvs_pool.tile([P, 8, 2, HD], bf16, name="kvs")
                    for t, w_sb in ((0, wk_sb), (1, wv_sb)):
                        ps = kv_ps.tile([P, 512], f32, name="ps_kv", tag="mm512")
                        for dc in range(NDC):
                            nc.tensor.matmul(
                                ps[:],
                                xt_sb[:, dc, sc * P : (sc + 1) * P],
                                w_sb[:, dc, wcol : wcol + 512],
                                start=(dc == 0),
                                stop=(dc == NDC - 1),
                            )
                        if include_biases:
                            nc.vector.tensor_tensor(
                                out=ps[:],
                                in0=ps[:],
                                in1=bkv_bc[:, t, wcol : wcol + 512],
                                op=mybir.AluOpType.add,
                            )
                        nc.vector.tensor_scalar(
                            out=kvs[:, :, t, :],
                            in0=ps[:],
                            scalar1=m_sb[:, sc : sc + 1],
                            scalar2=None,
                            op0=mybir.AluOpType.mult,
                        )
                    for hp in range(4):
                        for i in range(2):
                            h = 2 * hp + i
                            nc.tensor.matmul(
                                kp_acc[hp][:, i, :],
                                kvs[:, h, :, :],
                                eT[:, h, :],
                                start=(sc == 0 and i == 0),
                                stop=(sc == NSC - 1 and i == 1),
                                skip_group_check=True,
                            )

            def ship_partials(kp_acc, cc_in):
                for hp in range(4):
                    kp_sb = kp_pool.tile([P, 2, LK], bf16, name="kp_sb")
                    nc.vector.tensor_copy(kp_sb[:], kp_acc[hp][:])
                    for i in range(2):
                        nc.scalar.dma_start(out=cc_in[2 * hp + i], in_=kp_sb[:, i, :])

            # ---------------- pass A: heads 0-7 ----------------
            psA_scope = ExitStack()
            kpA_ps = psA_scope.enter_context(
                tc.tile_pool(name="kpA_ps", bufs=1, space="PSUM")
            )
            kpA = [kpA_ps.tile([P, 2, LK], f32, name=f"kpA{i}") for i in range(4)]
            kv_pass(
                EA_e, 0, kpA,
                extra={10: lambda: nc.sync.dma_start(wq_sb[:], WQ_e.ap())},
            )
            ship_partials(kpA, cc_in_a)
            psA_scope.close()
            nc.gpsimd.collective_compute(
                "AllReduce",
                mybir.AluOpType.add,
                replica_groups=PAIRS,
                ins=[cc_in_a[:].opt()],
                outs=[cc_out_a[:].opt()],
            )
            # readback A rides the ACT HWDGE ring while pass B computes
            vpTA_scope = ExitStack()
            vpTA_pool = vpTA_scope.enter_context(
                tc.tile_pool(name="vpTA", bufs=8, side="right")
            )
            vpTA = []
            for hl in range(8):
                par = (hl % 2) * 64
                nc.scalar.dma_start(
                    out=kpT[par : par + 64, hl // 2, :], in_=cc_out_a[hl, 0:64, :]
                )
                vpT_sb = vpTA_pool.tile([64, 2, P], bf16, name="vpT_sb")
                nc.scalar.dma_start(out=vpT_sb[:], in_=cc_out_a[hl, 64:128, :])
                vpTA.append((hl, vpT_sb))

            # ---------------- pass B: heads 8-15 ----------------
            psB_scope = ExitStack()
            kpB_ps = psB_scope.enter_context(
                tc.tile_pool(name="kpB_ps", bufs=1, space="PSUM")
            )
            kpB = [kpB_ps.tile([P, 2, LK], f32, name=f"kpB{i}") for i in range(4)]
            kv_pass(EB_e, 512, kpB)
            ship_partials(kpB, cc_in_b)
            psB_scope.close()
            ps_scope.close()
            nc.gpsimd.collective_compute(
                "AllReduce",
                mybir.AluOpType.add,
                replica_groups=PAIRS,
                ins=[cc_in_b[:].opt()],
                outs=[cc_out_b[:].opt()],
            )

            # vp transposes for group A (PE work, fits between Q groups)
            psT_scope = ExitStack()
            tp_ps_pool = psT_scope.enter_context(
                tc.tile_pool(name="tp_ps", bufs=2, space="PSUM")
            )
            q_ps_pool = psT_scope.enter_context(
                tc.tile_pool(name="q_ps", bufs=4, space="PSUM")
            )

            def vp_transpose(h, vpT_sb):
                for c in range(2):
                    tp_ps = tp_ps_pool.tile([P, HD], bf16, name="tp_ps")
                    nc.tensor.transpose(tp_ps[:], vpT_sb[:, c, :], id_sb[0:64, 0:64])
                    nc.vector.tensor_copy(vp_sb2[:, h, c, :], tp_ps[:])

            for hl, vpT_sb in vpTA:
                vp_transpose(hl, vpT_sb)

            # readback B DMAs (ACT ring, gated on AllReduce B)
            vpTB = []
            for hl in range(8):
                h = 8 + hl
                par = (h % 2) * 64
                nc.scalar.dma_start(
                    out=kpT[par : par + 64, h // 2, :], in_=cc_out_b[hl, 0:64, :]
                )
                vpT_sb = vpTA_pool.tile([64, 2, P], bf16, name="vpT_sb")
                nc.scalar.dma_start(out=vpT_sb[:], in_=cc_out_b[hl, 64:128, :])
                vpTB.append((h, vpT_sb))

            # ---------------- Q projection (covers AllReduce B) ----------------
            for mc in range(NDC):
                for sn in range(NSN):
                    ps = q_ps_pool.tile([P, 512], f32, name="psq")
                    for dc in range(NDC):
                        nc.tensor.matmul(
                            ps[:],
                            wq_sb[:, dc, mc * P : (mc + 1) * P],
                            xt_sb[:, dc, sn * 512 : (sn + 1) * 512],
                            start=(dc == 0),
                            stop=(dc == NDC - 1),
                        )
                    nc.vector.tensor_scalar(
                        out=qT[:, mc, sn * 512 : (sn + 1) * 512],
                        in0=ps[:],
                        scalar1=bq_sb[:, mc : mc + 1],
                        scalar2=None,
                        op0=mybir.AluOpType.add,
                    )

            for h, vpT_sb in vpTB:
                vp_transpose(h, vpT_sb)
            psT_scope.close()
            vpTA_scope.close()
            kvs_scope.close()
            e_scope.close()
            kp_scope.close()
            free_wv()
            free_wk()
            free_xt()

            # ---------------- attention + inline output projection --------------
            wo_sb, free_wo = tc.tile([P, NDC, D], bf16, name="wo_sb")
            nc.sync.dma_start(wo_sb[:], WO_e.ap())
            xoT, free_xoT = tc.tile([P, NDC, SL], bf16, name="xoT")

            with (
                tc.tile_pool(name="at_pool", bufs=3, side="right") as at_pool,
                tc.tile_pool(name="rbc_pool", bufs=2, side="right") as rbc_pool,
                tc.tile_pool(name="osb_pool", bufs=3, side="right") as osb_pool,
                tc.tile_pool(name="ps_dot", bufs=4, space="PSUM") as ps_dot,
                tc.tile_pool(name="ps_xoden", bufs=2, space="PSUM") as ps_xoden,
                tc.tile_pool(name="ps_out", bufs=2, space="PSUM") as ps_out,
            ):
                def attn_dot(sn, j):
                    # heads (2j, 2j+1): even head on partitions 0-63, odd on 64-127
                    ssl = slice(sn * 512, (sn + 1) * 512)
                    ats = []
                    dps = {}
                    for kc in range(2):
                        for pi, par in ((0, 0), (1, 64)):
                            d = ps_dot.tile([P, 512], f32, name="dps")
                            nc.tensor.matmul(
                                d[:],
                                kpT[par : par + 64, j, kc * P : (kc + 1) * P],
                                qT[par : par + 64, j, ssl],
                                start=True,
                                stop=True,
                            )
                            dps[(kc, pi)] = d
                    for pi in range(2):
                        at = at_pool.tile([P, 2, 512], bf16, name="at")
                        for kc in range(2):
                            nc.scalar.activation(
                                out=at[:, kc, :],
                                in_=dps[(kc, pi)][:],
                                func=mybir.ActivationFunctionType.Exp,
                            )
                        ats.append(at)
                    return ats

                def attn_pv(sn, j, ats):
                    ssl = slice(sn * 512, (sn + 1) * 512)
                    xo_ps = ps_xoden.tile([P, 512], f32, name="xo_ps", tag="xoden")
                    den_ps = ps_xoden.tile([P, 512], f32, name="den_ps", tag="xoden")
                    for kc in range(2):
                        for pi, par in ((0, 0), (1, 64)):
                            h = 2 * j + pi
                            nc.tensor.matmul(
                                xo_ps[par : par + 64, :],
                                vp_sb2[:, h, kc, :],
                                ats[pi][:, kc, :],
                                start=(kc == 0),
                                stop=(kc == 1),
                                skip_group_check=True,
                            )
                            nc.tensor.matmul(
                                den_ps[par : par + 64, :],
                                ones_sb[:],
                                ats[pi][:, kc, :],
                                start=(kc == 0),
                                stop=(kc == 1),
                                skip_group_check=True,
                            )
                    rbc = rbc_pool.tile([P, 512], f32, name="rbc")
                    nc.vector.reciprocal_approx_fast(out=rbc[:], in_=den_ps[:])
                    nc.vector.tensor_tensor(
                        out=xoT[:, j, ssl],
                        in0=xo_ps[:],
                        in1=rbc[:],
                        op=mybir.AluOpType.mult,
                    )

                def outproj(sn):
                    for si in range(4):
                        sc = sn * 4 + si
                        for half in range(2):
                            ps = ps_out.tile([P, 512], f32, name="ps_o")
                            for c in range(NDC):
                                nc.tensor.matmul(
                                    ps[:],
                                    xoT[:, c, sc * P : (sc + 1) * P],
                                    wo_sb[:, c, half * 512 : (half + 1) * 512],
                                    start=(c == 0),
                                    stop=(c == NDC - 1),
                                )
                            osb = osb_pool.tile([P, 512], f32, name="osb")
                            nc.vector.tensor_tensor(
                                out=osb[:],
                                in0=ps[:],
                                in1=bo_bc[:, half * 512 : (half + 1) * 512],
                                op=mybir.AluOpType.add,
                            )
                            eng = nc.sync if half == 0 else nc.scalar
                            eng.dma_start(
                                out=out_e[
                                    sc * P : (sc + 1) * P, half * 512 : (half + 1) * 512
                                ],
                                in_=osb[:],
                            )

                # software-pipelined by one iteration: dot(i+1) is emitted before
                # pv(i) so the PE FIFO never stalls on the exp of the current tile
                iters = [(sn, j) for sn in range(NSN) for j in range(H // 2)]
                pending = None
                for sn, j in iters:
                    ats = attn_dot(sn, j)
                    if pending is not None:
                        psn, pj, pats = pending
                        attn_pv(psn, pj, pats)
                        if pj == H // 2 - 1:
                            outproj(psn)
                    pending = (sn, j, ats)
                psn, pj, pats = pending
                attn_pv(psn, pj, pats)
                outproj(psn)

            if debug:
                nc.gpsimd.dma_start(out=dbg_cca[:], in_=cc_in_a[:])
                nc.gpsimd.dma_start(out=dbg_ccb[:], in_=cc_in_b[:])
                nc.gpsimd.dma_start(out=dbg_qT[:], in_=qT[:])
                nc.gpsimd.dma_start(out=dbg_kpT[:], in_=kpT[:])
                nc.gpsimd.dma_start(out=dbg_vp[:], in_=vp_sb2[:])
                nc.gpsimd.dma_start(out=dbg_xoT[:], in_=xoT[:])
            free_xoT()
            free_wo()
            free_wq()
            free_qT()
            free_vp()
            free_kpT()

    nc.compile()
    return nc


_cache = {}


def _get_nc(include_biases: bool, debug: bool = False):
    key = (include_biases, debug)
    if key not in _cache:
        _cache[key] = _build(include_biases, debug)
    return _cache[key]


def prepare_in_maps(inputs):
    X = np.asarray(inputs["X"], np.float32)
    mask = np.asarray(inputs["mask"], np.float32)
    E = np.asarray(inputs["E"], np.float32)
    Ws = {k: np.asarray(inputs[k], np.float32) for k in ("Wq", "Wk", "Wv", "Wo")}
    bs = {k: np.asarray(inputs[k], np.float32) for k in ("bq", "bk", "bv", "bo")}

    include_biases = bool(np.any(bs["bk"]) or np.any(bs["bv"]))

    def wprep(w):
        return np.ascontiguousarray(
            w.reshape(NDC, P, D).transpose(1, 0, 2)
        ).astype(BF16)

    WK = wprep(Ws["Wk"])
    WV = wprep(Ws["Wv"])
    WQ = wprep(Ws["Wq"] * 0.125)
    WO = wprep(Ws["Wo"])
    BQ = np.ascontiguousarray((bs["bq"] * 0.125).reshape(NDC, P).T)
    BO = bs["bo"]
    BKV = np.stack([bs["bk"], bs["bv"]])

    # E^T once: [S, H, LK]
    ET = np.ascontiguousarray(E.transpose(2, 0, 1))
    E_half = {}
    for half in range(2):
        sl = slice(half * SL, (half + 1) * SL)
        Eh = ET[sl].astype(BF16)  # [SL, H, LK]
        EA = np.ascontiguousarray(Eh[:, 0:8, :]).reshape(NSC, P, 8, LK)
        EB = np.ascontiguousarray(Eh[:, 8:16, :]).reshape(NSC, P, 8, LK)
        E_half[half] = (EA, EB)

    in_maps = []
    for c in range(8):
        b, half = c // 2, c % 2
        sl = slice(half * SL, (half + 1) * SL)
        XT = np.ascontiguousarray(
            X[b, sl, :].T.reshape(NDC, P, SL).transpose(1, 0, 2)
        ).astype(BF16)
        MS = np.ascontiguousarray(mask[b, sl].reshape(NSC, P).T)
        EA, EB = E_half[half]
        m = {
            "XT": XT, "EA": EA, "EB": EB,
            "WK": WK, "WV": WV, "WQ": WQ, "WO": WO,
            "BQ": BQ, "MS": MS, "BO": BO,
        }
        if include_biases:
            m["BKV"] = BKV
        in_maps.append(m)
    return include_biases, in_maps


def kernel(**inputs) -> np.ndarray:
    include_biases, in_maps = prepare_in_maps(inputs)
    nc = _get_nc(include_biases)
    res = bass_utils.run_bass_kernel_spmd(nc, in_maps, core_ids=list(range(8)))
    out = np.empty((B, S, D), np.float32)
    for c in range(8):
        b, half = c // 2, c % 2
        out[b, half * SL : (half + 1) * SL, :] = res.results[c]["out"]
    return out


# revision 10
# speedup vs baseline: 1.6154x; 1.0316x over previous
"""Trainium2 Bass kernel for Linformer-style sparse attention.

Problem shapes (hardcoded): B=4, S=4096, D=1024, H=16, HD=64, LK=256.

Sharding (8 cores): core c -> (batch b = c//2, sequence half = c%2).
Each core:
  - computes Q/K/V for its 2048 rows (all heads),
  - computes partial [Kp^T; Vp^T] = (K|V)^T @ E^T over its rows,
  - pair AllReduce ([0,1],[2,3],[4,5],[6,7]) completes Kp/Vp,
  - attention (softmax over LK=256) + output projection for its own rows,
  - writes its [2048, 1024] slice of the output directly (no final collective).

All inputs are pre-transposed and pre-cast to bf16 on the host in exact
SBUF layouts, so the device does no transposes or cast round-trips:
  XT [128, 8, 2048]       X^T with d=dc*128+p
  EA/EB [16, 128, 8, 256] E^T per seq-chunk for head groups 0-7 / 8-15
  WK/WV/WQ/WO [128, 8, 1024] with d_in = o*128+p; WQ pre-scaled by 1/sqrt(HD)

Schedule: the K/V projection runs in two per-head-group passes, each fused
with its Kp/Vp partial accumulation (PSUM accumulators across the 16
seq-chunks, E^T streaming in 0.5 MiB chunks).  AllReduce A kicks at ~50%
of the KV work and hides under pass B; AllReduce B hides under the Q
projection.  Readback DMAs ride the ACT HWDGE ring (the GpSimd SWDGE
desc-gen at ~0.6us each would serialize).  The attention loop is
software-pipelined one iteration ahead so the PE FIFO never waits on exp.
"""

import sys

sys.path.insert(0, "/opt/trn_rl_repo")

from contextlib import ExitStack

import numpy as np
import ml_dtypes

from concourse import bacc, bass_utils, mybir, tile
from concourse.masks import make_identity

B, S, D = 4, 4096, 1024
H, HD, LK = 16, 64, 256
SL = S // 2            # local sequence rows per core
P = 128
NSC = SL // P          # 16 s-chunks of 128
NDC = D // P           # 8 d-chunks of 128
NSN = SL // 512        # 4 s-blocks of 512
f32 = mybir.dt.float32
bf16 = mybir.dt.bfloat16
PAIRS = [[0, 1], [2, 3], [4, 5], [6, 7]]
BF16 = ml_dtypes.bfloat16


def _build(include_biases: bool, debug: bool = False):
    nc = bacc.Bacc("TRN2", target_bir_lowering=False, num_devices=8)

    XT_e = nc.declare_dram_parameter("XT", [P, NDC, SL], bf16, isOutput=False)
    EA_e = nc.declare_dram_parameter("EA", [NSC, P, 8, LK], bf16, isOutput=False)
    EB_e = nc.declare_dram_parameter("EB", [NSC, P, 8, LK], bf16, isOutput=False)
    WK_e = nc.declare_dram_parameter("WK", [P, NDC, D], bf16, isOutput=False)
    WV_e = nc.declare_dram_parameter("WV", [P, NDC, D], bf16, isOutput=False)
    WQ_e = nc.declare_dram_parameter("WQ", [P, NDC, D], bf16, isOutput=False)
    WO_e = nc.declare_dram_parameter("WO", [P, NDC, D], bf16, isOutput=False)
    BQ_e = nc.declare_dram_parameter("BQ", [P, NDC], f32, isOutput=False)
    MS_e = nc.declare_dram_parameter("MS", [P, NSC], f32, isOutput=False)
    BO_e = nc.declare_dram_parameter("BO", [D], f32, isOutput=False)
    if include_biases:
        BKV_e = nc.declare_dram_parameter("BKV", [2, D], f32, isOutput=False)
    out_e = nc.declare_dram_parameter("out", [SL, D], f32, isOutput=True)
    if debug:
        dbg_cca = nc.declare_dram_parameter("dbg_cca", [8, P, LK], f32, isOutput=True)
        dbg_ccb = nc.declare_dram_parameter("dbg_ccb", [8, P, LK], f32, isOutput=True)
        dbg_qT = nc.declare_dram_parameter("dbg_qT", [P, NDC, SL], f32, isOutput=True)
        dbg_kpT = nc.declare_dram_parameter("dbg_kpT", [P, H // 2, LK], f32, isOutput=True)
        dbg_vp = nc.declare_dram_parameter("dbg_vp", [P, H, 2, HD], f32, isOutput=True)
        dbg_xoT = nc.declare_dram_parameter("dbg_xoT", [P, NDC, SL], f32, isOutput=True)

    # AllReduce bounce (bf16): per head [KpT ; VpT] stacked [128, 256]
    cc_in_a = nc.dram_tensor("cc_in_a", [8, P, LK], bf16, kind="Internal")
    cc_out_a = nc.dram_tensor("cc_out_a", [8, P, LK], bf16, kind="Internal")
    cc_in_b = nc.dram_tensor("cc_in_b", [8, P, LK], bf16, kind="Internal")
    cc_out_b = nc.dram_tensor("cc_out_b", [8, P, LK], bf16, kind="Internal")

    with tile.TileContext(nc) as tc:
        ctx = ExitStack()
        with ctx:
            const_pool = ctx.enter_context(tc.tile_pool(name="consts", bufs=1))

            # ---------------- constants ----------------
            m_sb = const_pool.tile([P, NSC], f32, name="m_sb")
            nc.scalar.dma_start(m_sb[:], MS_e.ap())
            bq_sb = const_pool.tile([P, NDC], f32, name="bq_sb")
            nc.scalar.dma_start(bq_sb[:], BQ_e.ap())
            bo_bc = const_pool.tile([P, D], f32, name="bo_bc")
            nc.gpsimd.dma_start(out=bo_bc[:], in_=BO_e.ap()[None, :].to_broadcast((P, D)))
            if include_biases:
                bkv_bc = const_pool.tile([P, 2, D], f32, name="bkv_bc")
                nc.gpsimd.dma_start(
                    out=bkv_bc[:, 0, :], in_=BKV_e.ap()[0][None, :].to_broadcast((P, D))
                )
                nc.gpsimd.dma_start(
                    out=bkv_bc[:, 1, :], in_=BKV_e.ap()[1][None, :].to_broadcast((P, D))
                )
            id_sb = const_pool.tile([P, P], bf16, name="id_sb")
            make_identity(nc, id_sb[:])
            ones_sb = const_pool.tile([P, HD], bf16, name="ones_sb")
            nc.vector.memset(ones_sb[:], 1.0)

            # ------------- persistent tiles (left stack; frees must be LIFO) ----
            kpT, free_kpT = tc.tile([P, H // 2, LK], bf16, name="kpT")
            vp_sb2, free_vp = tc.tile([P, H, 2, HD], bf16, name="vp_sb2")
            qT, free_qT = tc.tile([P, NDC, SL], bf16, name="qT")
            wq_sb, free_wq = tc.tile([P, NDC, D], bf16, name="wq_sb")
            xt_sb, free_xt = tc.tile([P, NDC, SL], bf16, name="xt_sb")
            wk_sb, free_wk = tc.tile([P, NDC, D], bf16, name="wk_sb")
            wv_sb, free_wv = tc.tile([P, NDC, D], bf16, name="wv_sb")

            # prologue loads, ordered so the first KV matmuls unblock early;
            # xt chunks 1-3 stream during early pass A (chunk q feeds sc>=4q)
            def xt_chunk(q):
                nc.sync.dma_start(
                    xt_sb[:, :, q * 512 : (q + 1) * 512],
                    XT_e.ap()[:, :, q * 512 : (q + 1) * 512],
                )

            nc.sync.dma_start(wk_sb[:], WK_e.ap())
            xt_chunk(0)
            nc.sync.dma_start(wv_sb[:], WV_e.ap())

            # transient pools (right stack)
            kp_scope = ExitStack()
            kp_pool = kp_scope.enter_context(
                tc.tile_pool(name="kp_sb", bufs=4, side="right")
            )
            e_scope = ExitStack()
            e_pool = e_scope.enter_context(
                tc.tile_pool(name="e_pool", bufs=6, side="right")
            )
            kvs_scope = ExitStack()
            kvs_pool = kvs_scope.enter_context(
                tc.tile_pool(name="kvs", bufs=3, side="right")
            )

            ps_scope = ExitStack()
            kv_ps = ps_scope.enter_context(
                tc.tile_pool(name="kv_ps", bufs=4, space="PSUM")
            )

            def kv_pass(E_param, wcol, kp_acc, extra=None):
                # K/V projection for one head group (8 heads = weight columns
                # [wcol, wcol+512)) fused with the Kp/Vp partial accumulation.
                for sc in range(NSC):
                    eT = e_pool.tile([P, 8, LK], bf16, name="eT")
                    nc.sync.dma_start(eT[:], E_param[sc])
                    if extra is not None and sc in extra:
                        extra[sc]()
                    kvs = kvs_pool.tile([P, 8, 2, HD], bf16, name="kvs")
                    for t, w_sb in ((0, wk_sb), (1, wv_sb)):
                        ps = kv_ps.tile([P, 512], f32, name="ps_kv", tag="mm512")
                        for dc in range(NDC):
                            nc.tensor.matmul(
                                ps[:],
                                xt_sb[:, dc, sc * P : (sc + 1) * P],
                                w_sb[:, dc, wcol : wcol + 512],
                                start=(dc == 0),
                                stop=(dc == NDC - 1),
                            )
                        if include_biases:
                            nc.vector.tensor_tensor(
                                out=ps[:],
                                in0=ps[:],
                                in1=bkv_bc[:, t, wcol : wcol + 512],
                                op=mybir.AluOpType.add,
                            )
                        nc.vector.tensor_scalar(
                            out=kvs[:, :, t, :],
                            in0=ps[:],
                            scalar1=m_sb[:, sc : sc + 1],
                            scalar2=None,
                            op0=mybir.AluOpType.mult,
                        )
                    for hp in range(4):
                        for i in range(2):
                            h = 2 * hp + i
                            nc.tensor.matmul(
                                kp_acc[hp][:, i, :],
                                kvs[:, h, :, :],
                                eT[:, h, :],
                                start=(sc == 0 and i == 0),
                                stop=(sc == NSC - 1 and i == 1),
                                skip_group_check=True,
                            )

            def ship_partials(kp_acc, cc_in):
                for hp in range(4):
                    kp_sb = kp_pool.tile([P, 2, LK], bf16, name="kp_sb")
                    nc.vector.tensor_copy(kp_sb[:], kp_acc[hp][:])
                    for i in range(2):
                        nc.scalar.dma_start(out=cc_in[2 * hp + i], in_=kp_sb[:, i, :])

            # ---------------- pass A: heads 0-7 ----------------
            psA_scope = ExitStack()
            kpA_ps = psA_scope.enter_context(
                tc.tile_pool(name="kpA_ps", bufs=1, space="PSUM")
            )
            kpA = [kpA_ps.tile([P, 2, LK], f32, name=f"kpA{i}") for i in range(4)]
            kv_pass(
                EA_e, 0, kpA,
                extra={
                    0: lambda: xt_chunk(1),
                    1: lambda: xt_chunk(2),
                    2: lambda: xt_chunk(3),
                    10: lambda: nc.sync.dma_start(wq_sb[:], WQ_e.ap()),
                },
            )
            ship_partials(kpA, cc_in_a)
            psA_scope.close()
            nc.gpsimd.collective_compute(
                "AllReduce",
                mybir.AluOpType.add,
                replica_groups=PAIRS,
                ins=[cc_in_a[:].opt()],
                outs=[cc_out_a[:].opt()],
            )
            # readback A rides the ACT HWDGE ring while pass B computes
            vpTA_scope = ExitStack()
            vpTA_pool = vpTA_scope.enter_context(
                tc.tile_pool(name="vpTA", bufs=8, side="right")
            )
            vpTA = []
            for hl in range(8):
                par = (hl % 2) * 64
                nc.scalar.dma_start(
                    out=kpT[par : par + 64, hl // 2, :], in_=cc_out_a[hl, 0:64, :]
                )
                vpT_sb = vpTA_pool.tile([64, 2, P], bf16, name="vpT_sb")
                nc.scalar.dma_start(out=vpT_sb[:], in_=cc_out_a[hl, 64:128, :])
                vpTA.append((hl, vpT_sb))

            # ---------------- pass B: heads 8-15 ----------------
            psB_scope = ExitStack()
            kpB_ps = psB_scope.enter_context(
                tc.tile_pool(name="kpB_ps", bufs=1, space="PSUM")
            )
            kpB = [kpB_ps.tile([P, 2, LK], f32, name=f"kpB{i}") for i in range(4)]
            kv_pass(EB_e, 512, kpB)
            ship_partials(kpB, cc_in_b)
            psB_scope.close()
            ps_scope.close()
            nc.gpsimd.collective_compute(
                "AllReduce",
                mybir.AluOpType.add,
                replica_groups=PAIRS,
                ins=[cc_in_b[:].opt()],
                outs=[cc_out_b[:].opt()],
            )

            # vp transposes for group A (PE work, fits between Q groups)
            psT_scope = ExitStack()
            tp_ps_pool = psT_scope.enter_context(
                tc.tile_pool(name="tp_ps", bufs=2, space="PSUM")
            )
            q_ps_pool = psT_scope.enter_context(
                tc.tile_pool(name="q_ps", bufs=4, space="PSUM")
            )

            def vp_transpose(h, vpT_sb):
                for c in range(2):
                    tp_ps = tp_ps_pool.tile([P, HD], bf16, name="tp_ps")
                    nc.tensor.transpose(tp_ps[:], vpT_sb[:, c, :], id_sb[0:64, 0:64])
                    nc.vector.tensor_copy(vp_sb2[:, h, c, :], tp_ps[:])

            for hl, vpT_sb in vpTA:
                vp_transpose(hl, vpT_sb)

            # readback B DMAs (ACT ring, gated on AllReduce B)
            vpTB = []
            for hl in range(8):
                h = 8 + hl
                par = (h % 2) * 64
                nc.scalar.dma_start(
                    out=kpT[par : par + 64, h // 2, :], in_=cc_out_b[hl, 0:64, :]
                )
                vpT_sb = vpTA_pool.tile([64, 2, P], bf16, name="vpT_sb")
                nc.scalar.dma_start(out=vpT_sb[:], in_=cc_out_b[hl, 64:128, :])
                vpTB.append((h, vpT_sb))

            # ---------------- Q projection (covers AllReduce B) ----------------
            for mc in range(NDC):
                for sn in range(NSN):
                    ps = q_ps_pool.tile([P, 512], f32, name="psq")
                    for dc in range(NDC):
                        nc.tensor.matmul(
                            ps[:],
                            wq_sb[:, dc, mc * P : (mc + 1) * P],
                            xt_sb[:, dc, sn * 512 : (sn + 1) * 512],
                            start=(dc == 0),
                            stop=(dc == NDC - 1),
                        )
                    nc.vector.tensor_scalar(
                        out=qT[:, mc, sn * 512 : (sn + 1) * 512],
                        in0=ps[:],
                        scalar1=bq_sb[:, mc : mc + 1],
                        scalar2=None,
                        op0=mybir.AluOpType.add,
                    )

            for h, vpT_sb in vpTB:
                vp_transpose(h, vpT_sb)
            psT_scope.close()
            vpTA_scope.close()
            kvs_scope.close()
            e_scope.close()
            kp_scope.close()
            free_wv()
            free_wk()
            free_xt()

            # ---------------- attention + inline output projection --------------
            wo_sb, free_wo = tc.tile([P, NDC, D], bf16, name="wo_sb")
            nc.sync.dma_start(wo_sb[:], WO_e.ap())
            xoT, free_xoT = tc.tile([P, NDC, SL], bf16, name="xoT")

            with (
                tc.tile_pool(name="at_pool", bufs=3, side="right") as at_pool,
                tc.tile_pool(name="rbc_pool", bufs=2, side="right") as rbc_pool,
                tc.tile_pool(name="osb_pool", bufs=3, side="right") as osb_pool,
                tc.tile_pool(name="ps_dot", bufs=4, space="PSUM") as ps_dot,
                tc.tile_pool(name="ps_xoden", bufs=2, space="PSUM") as ps_xoden,
                tc.tile_pool(name="ps_out", bufs=2, space="PSUM") as ps_out,
            ):
                def attn_dot(sn, j):
                    # heads (2j, 2j+1): even head on partitions 0-63, odd on 64-127
                    ssl = slice(sn * 512, (sn + 1) * 512)
                    ats = []
                    dps = {}
                    for kc in range(2):
                        for pi, par in ((0, 0), (1, 64)):
                            d = ps_dot.tile([P, 512], f32, name="dps")
                            nc.tensor.matmul(
                                d[:],
                                kpT[par : par + 64, j, kc * P : (kc + 1) * P],
                                qT[par : par + 64, j, ssl],
                                start=True,
                                stop=True,
                            )
                            dps[(kc, pi)] = d
                    for pi in range(2):
                        at = at_pool.tile([P, 2, 512], bf16, name="at")
                        for kc in range(2):
                            nc.scalar.activation(
                                out=at[:, kc, :],
                                in_=dps[(kc, pi)][:],
                                func=mybir.ActivationFunctionType.Exp,
                            )
                        ats.append(at)
                    return ats

                def attn_pv(sn, j, ats):
                    ssl = slice(sn * 512, (sn + 1) * 512)
                    xo_ps = ps_xoden.tile([P, 512], f32, name="xo_ps", tag="xoden")
                    den_ps = ps_xoden.tile([P, 512], f32, name="den_ps", tag="xoden")
                    for kc in range(2):
                        for pi, par in ((0, 0), (1, 64)):
                            h = 2 * j + pi
                            nc.tensor.matmul(
                                xo_ps[par : par + 64, :],
                                vp_sb2[:, h, kc, :],
                                ats[pi][:, kc, :],
                                start=(kc == 0),
                                stop=(kc == 1),
                                skip_group_check=True,
                            )
                            nc.tensor.matmul(
                                den_ps[par : par + 64, :],
                                ones_sb[:],
                                ats[pi][:, kc, :],
                                start=(kc == 0),
                                stop=(kc == 1),
                                skip_group_check=True,
                            )
                    rbc = rbc_pool.tile([P, 512], f32, name="rbc")
                    nc.vector.reciprocal_approx_fast(out=rbc[:], in_=den_ps[:])
                    nc.vector.tensor_tensor(
                        out=xoT[:, j, ssl],
                        in0=xo_ps[:],
                        in1=rbc[:],
                        op=mybir.AluOpType.mult,
                    )

                def outproj(sn):
                    for si in range(4):
                        sc = sn * 4 + si
                        osb = osb_pool.tile([P, D], f32, name="osb")
                        for half in range(2):
                            ps = ps_out.tile([P, 512], f32, name="ps_o")
                            for c in range(NDC):
                                nc.tensor.matmul(
                                    ps[:],
                                    xoT[:, c, sc * P : (sc + 1) * P],
                                    wo_sb[:, c, half * 512 : (half + 1) * 512],
                                    start=(c == 0),
                                    stop=(c == NDC - 1),
                                )
                            nc.vector.tensor_tensor(
                                out=osb[:, half * 512 : (half + 1) * 512],
                                in0=ps[:],
                                in1=bo_bc[:, half * 512 : (half + 1) * 512],
                                op=mybir.AluOpType.add,
                            )
                        eng = nc.sync if sc % 2 == 0 else nc.scalar
                        eng.dma_start(
                            out=out_e[sc * P : (sc + 1) * P, :], in_=osb[:]
                        )

                # software-pipelined by one iteration: dot(i+1) is emitted before
                # pv(i) so the PE FIFO never stalls on the exp of the current tile
                iters = [(sn, j) for sn in range(NSN) for j in range(H // 2)]
                pending = None
                for sn, j in iters:
                    ats = attn_dot(sn, j)
                    if pending is not None:
                        psn, pj, pats = pending
                        attn_pv(psn, pj, pats)
                        if pj == H // 2 - 1:
                            outproj(psn)
                    pending = (sn, j, ats)
                psn, pj, pats = pending
                attn_pv(psn, pj, pats)
                outproj(psn)

            if debug:
                nc.gpsimd.dma_start(out=dbg_cca[:], in_=cc_in_a[:])
                nc.gpsimd.dma_start(out=dbg_ccb[:], in_=cc_in_b[:])
                nc.gpsimd.dma_start(out=dbg_qT[:], in_=qT[:])
                nc.gpsimd.dma_start(out=dbg_kpT[:], in_=kpT[:])
                nc.gpsimd.dma_start(out=dbg_vp[:], in_=vp_sb2[:])
                nc.gpsimd.dma_start(out=dbg_xoT[:], in_=xoT[:])
            free_xoT()
            free_wo()
            free_wq()
            free_qT()
            free_vp()
            free_kpT()

    nc.compile()
    return nc


_cache = {}


def _get_nc(include_biases: bool, debug: bool = False):
    key = (include_biases, debug)
    if key not in _cache:
        _cache[key] = _build(include_biases, debug)
    return _cache[key]


def prepare_in_maps(inputs):
    X = np.asarray(inputs["X"], np.float32)
    mask = np.asarray(inputs["mask"], np.float32)
    E = np.asarray(inputs["E"], np.float32)
    Ws = {k: np.asarray(inputs[k], np.float32) for k in ("Wq", "Wk", "Wv", "Wo")}
    bs = {k: np.asarray(inputs[k], np.float32) for k in ("bq", "bk", "bv", "bo")}

    include_biases = bool(np.any(bs["bk"]) or np.any(bs["bv"]))

    def wprep(w):
        return np.ascontiguousarray(
            w.reshape(NDC, P, D).transpose(1, 0, 2)
        ).astype(BF16)

    WK = wprep(Ws["Wk"])
    WV = wprep(Ws["Wv"])
    WQ = wprep(Ws["Wq"] * 0.125)
    WO = wprep(Ws["Wo"])
    BQ = np.ascontiguousarray((bs["bq"] * 0.125).reshape(NDC, P).T)
    BO = bs["bo"]
    BKV = np.stack([bs["bk"], bs["bv"]])

    # E^T once: [S, H, LK]
    ET = np.ascontiguousarray(E.transpose(2, 0, 1))
    E_half = {}
    for half in range(2):
        sl = slice(half * SL, (half + 1) * SL)
        Eh = ET[sl].astype(BF16)  # [SL, H, LK]
        EA = np.ascontiguousarray(Eh[:, 0:8, :]).reshape(NSC, P, 8, LK)
        EB = np.ascontiguousarray(Eh[:, 8:16, :]).reshape(NSC, P, 8, LK)
        E_half[half] = (EA, EB)

    in_maps = []
    for c in range(8):
        b, half = c // 2, c % 2
        sl = slice(half * SL, (half + 1) * SL)
        XT = np.ascontiguousarray(
            X[b, sl, :].T.reshape(NDC, P, SL).transpose(1, 0, 2)
        ).astype(BF16)
        MS = np.ascontiguousarray(mask[b, sl].reshape(NSC, P).T)
        EA, EB = E_half[half]
        m = {
            "XT": XT, "EA": EA, "EB": EB,
            "WK": WK, "WV": WV, "WQ": WQ, "WO": WO,
            "BQ": BQ, "MS": MS, "BO": BO,
        }
        if include_biases:
            m["BKV"] = BKV
        in_maps.append(m)
    return include_biases, in_maps


def kernel(**inputs) -> np.ndarray:
    include_biases, in_maps = prepare_in_maps(inputs)
    nc = _get_nc(include_biases)
    res = bass_utils.run_bass_kernel_spmd(nc, in_maps, core_ids=list(range(8)))
    out = np.empty((B, S, D), np.float32)
    for c in range(8):
        b, half = c // 2, c % 2
        out[b, half * SL : (half + 1) * SL, :] = res.results[c]["out"]
    return out
